# revision 26
# baseline (speedup 1.0000x reference)
"""Trainium2 Bass kernel for gated multi-head attention with pair bias.

Reference computation (B=2, S=2048, C_IN=512, H=8, C=64):
    q,k,v = heads(x @ Wq), heads(x @ Wk), heads(x @ Wv)
    logits = q k^T / sqrt(C) + bias + mask_offset
    attn   = softmax(logits)
    o      = attn @ v
    out    = (sigmoid(x @ Wg + bg) * concat(o)) @ Wo + bo

Sharding: 8 cores = 2 batches x 4 head-pairs. Core c handles batch c//4,
heads (2*(c%4), 2*(c%4)+1). Weights are sliced per-core on the host; each
core computes a partial output (sum over its two heads) and the host sums
4 partials per batch and adds bo.

Key-compaction: attention_mask kills ~half the keys exactly (exp(-1e9)=0
in fp32), so the host gathers only the unmasked key positions (padded to
a multiple of 256 -> KP). k/v projections, qk logits, exp, softmax and
attn@v all run on KP instead of S keys, and only the kept columns of the
pair bias are shipped. This is exact, not approximate.

Device math:
  - Transposed orientation throughout (feature dims on partitions).
  - softmax skips max-subtraction (logits are O(+-8)), and uses
    exp(qk) * exp(bias) with exp(bias^T) precomputed on the host in bf16
    (padded key columns get exp-bias 0, which also kills the pad keys).
  - The softmax denominator comes from a ones column appended to v, so
    attn@v produces [o_unnorm ; rowsum] in one accumulation; 1/rowsum is
    applied after the output projection where q sits on partitions.
  - exp(qk)*ebias multiplies are split between DVE and GPSIMD so neither
    engine paces the loop; ACT (exp) is the intended pace-setter.
  - All matmuls bf16 with fp32 PSUM accumulation.
"""

import sys
import threading

import numpy as np

sys.path.insert(0, "/opt/trn_rl_repo")

import ml_dtypes

import concourse.bass as bass
import concourse.tile as tile
from concourse import mybir
from concourse.bass_utils import run_bass_kernel_spmd

# ---------------------------------------------------------------------------
# This toolchain's walrus encodes at most ONE semaphore wait per Drain/CTRL
# instruction; Tile's end-of-kernel drain can carry several (one per DMA
# queue). Split them across a chain of single-wait drains.
# ---------------------------------------------------------------------------


_NOP_UID = [0]


def _split_multi_waits(nc):
    """Rewrite every instruction carrying >1 sem waits: keep one wait on the
    instruction, hoist the others onto same-engine NoOps inserted right
    before it (engine streams execute in order, so this is equivalent)."""
    for fn in nc.m.functions:
        for bb in fn.blocks:
            insts = list(bb.instructions)
            out = []
            changed = False
            for inst in insts:
                si = inst.sync_info
                if si is not None and len(si.on_wait) > 1:
                    changed = True
                    waits = list(si.on_wait)
                    si.on_wait = waits[:1]
                    for w in waits[1:]:
                        _NOP_UID[0] += 1
                        nop = mybir.InstNoOp(
                            name=f"waitsplit-{_NOP_UID[0]}",
                            engine=inst.engine,
                            ins=[],
                            outs=[],
                        )
                        nop.sync_info = mybir.SyncInfo(on_wait=[w], on_update=[])
                        out.append(nop)
                out.append(inst)
            if changed:
                bb.instructions = out


def _drain_and_barrier_split(self, tick_clock, wait_clock):
    from concourse.vector_clock import ScopedClock

    drain_inst = self.nc.sync.drain()
    wait_clock.add_sem_waits(
        drain_inst.ins, ScopedClock({None: tick_clock.global_clock})
    )
    si = drain_inst.ins.sync_info
    if si is not None and len(si.on_wait) > 1:
        extra = list(si.on_wait[1:])
        si.on_wait = list(si.on_wait[:1])
        for w in extra:
            d2 = self.nc.sync.drain()
            d2.ins.sync_info = mybir.SyncInfo(on_wait=[w], on_update=[])

    self.nc.all_engine_barrier()
    assert self.sems is not None
    popped = self.nc._tile_sem_poison_stack.pop()
    assert popped is self._sem_poison
    self.nc.clear_and_free_semaphores(list(self.sems.allocated().values()))
    self.nc.all_engine_barrier()

    _split_multi_waits(self.nc)


tile.TileContext._drain_and_barrier = _drain_and_barrier_split

BF16 = mybir.dt.bfloat16
F32 = mybir.dt.float32
NBF = ml_dtypes.bfloat16

B, S, C_IN, H, C = 2, 2048, 512, 8, 64
P = 128
NCI = C_IN // P  # 4 contraction chunks
QH = 1024  # q-half processed per section
NQT = S // P  # 16 q tiles (output projection)

Exp = mybir.ActivationFunctionType.Exp
Tanh = mybir.ActivationFunctionType.Tanh


def _build_nc(KP):
    """KP = padded compacted key count (multiple of 256)."""
    KT = KP // P  # key tiles
    nc = bass.Bass("TRN2")

    x_t = nc.dram_tensor("xt", [NCI, P, S], BF16, kind="ExternalInput")
    xk_t = nc.dram_tensor("xkt", [NCI, P, KP], BF16, kind="ExternalInput")
    w_q = nc.dram_tensor("wq", [NCI, P, P], BF16, kind="ExternalInput")
    w_k = nc.dram_tensor("wk", [NCI, P, P], BF16, kind="ExternalInput")
    w_g = nc.dram_tensor("wg", [NCI, P, P], BF16, kind="ExternalInput")
    w_v = nc.dram_tensor("wv", [NCI, P, P], BF16, kind="ExternalInput")
    bg_t = nc.dram_tensor("bgv", [P, 1], F32, kind="ExternalInput")
    eb_t = nc.dram_tensor("ebias", [2, KP, S], BF16, kind="ExternalInput")
    wo_t = nc.dram_tensor("wo", [P, C_IN], BF16, kind="ExternalInput")
    out_t = nc.dram_tensor("out", [S, C_IN], F32, kind="ExternalOutput")

    from contextlib import ExitStack

    from concourse.alu_op_type import AluOpType as Alu

    with tile.TileContext(nc) as tc, ExitStack() as ctx:
        const = ctx.enter_context(tc.tile_pool(name="const", bufs=1))
        drp = ctx.enter_context(tc.tile_pool(name="dram", bufs=2, space="DRAM"))

        # ---------------- persistent SBUF tiles + input DMA ----------------
        def load_w(dram):
            t = const.tile([P, NCI, P], BF16, tag=f"w{dram.name}", name=f"wsb_{dram.name}")
            nc.sync.dma_start(t[:], dram[:].rearrange("c p m -> p c m"))
            return t

        # k/v first: their projections start the earliest
        wk_sb, wv_sb = load_w(w_k), load_w(w_v)
        xk_sb = [const.tile([P, KP], BF16, tag=f"xk{i}", name=f"xksb{i}") for i in range(NCI)]
        for i in range(NCI):
            nc.sync.dma_start(xk_sb[i][:], xk_t[i][:])
        wq_sb, wg_sb = load_w(w_q), load_w(w_g)
        xt_sb = [const.tile([P, S], BF16, tag=f"xt{i}", name=f"xtsb{i}") for i in range(NCI)]
        # first q-half of x lands first so the q/g projections (and the first
        # attention section) aren't gated on the whole tensor
        for half in range(2):
            hs = slice(half * QH, (half + 1) * QH)
            for i in range(NCI):
                nc.sync.dma_start(xt_sb[i][:, hs], x_t[i][:, hs])
        bgv_sb = const.tile([P, 1], F32, tag="bgv")
        nc.sync.dma_start(bgv_sb[:], bg_t[:])
        wo_st = const.tile([P, C_IN], BF16, tag="wo")
        nc.sync.dma_start(wo_st[:], wo_t[:])

        from concourse.masks import make_identity

        ident = const.tile([P, P], BF16, tag="ident")
        make_identity(nc, ident[:])
        # matmul fodder for the HAM warm-up burst
        junk = const.tile([P, 512], BF16, tag="junk")
        nc.gpsimd.memset(junk[:], 0.0)

        qT = const.tile([P, S], BF16, tag="qT")
        kT = const.tile([P, KP], BF16, tag="kT")
        gT = const.tile([P, S], BF16, tag="gT")
        goun = const.tile([P, S], BF16, tag="goun")
        vm = [const.tile([P, 2 * (C + 1)], BF16, tag=f"vm{t}", name=f"vm{t}") for t in range(KT)]
        rrec = [const.tile([P, NQT], F32, tag=f"rrec{h}", name=f"rrec{h}") for h in range(2)]

        # exp(bias) tiles stream per (h, j, kt). Triggers are emitted with a
        # fixed lookahead interleaved into the section loops so the sync
        # queue order matches consumption order (emitting them all up front
        # would block the queue on the pool's WAR semaphores and trap the
        # rowsum round-trip DMAs behind them).
        ebp = ctx.enter_context(tc.tile_pool(name="ebp", bufs=5))
        eb_seq = [(h, j, kt) for j in range(2) for h in range(2) for kt in range(KT)]
        eb_tiles = {}
        EB_AHEAD = 3

        def emit_eb(i):
            if i >= len(eb_seq):
                return
            h, j, kt = eb_seq[i]
            t = ebp.tile([P, QH], BF16, tag="eb")
            nc.sync.dma_start(
                t[:], eb_t[h, kt * P : (kt + 1) * P, j * QH : (j + 1) * QH]
            )
            eb_tiles[(h, j, kt)] = t

        for i in range(EB_AHEAD):
            emit_eb(i)

        # ---------------- projections ----------------
        with tc.tile_pool(name="projp", bufs=2, space="PSUM") as projp:
            # HAM warm-up: stream matmuls from t=0 (junk data, result
            # discarded) so the PE clock gate reaches 2.4 GHz before the
            # first real projection instead of ~3.4us into it. 10 MMs
            # cover the initial input-DMA wait.
            scr = projp.tile([P, 512], F32, tag="scr")
            for _ in range(6):
                nc.tensor.matmul(scr[:], junk[:, 0:P], junk[:], start=True, stop=True)
            # k projection over KP columns
            for ch in range(KP // 512):
                pp = projp.tile([P, 512], F32, tag="pp")
                sl = slice(ch * 512, (ch + 1) * 512)
                for ci in range(NCI):
                    nc.tensor.matmul(
                        pp[:], wk_sb[:, ci, :], xk_sb[ci][:, sl],
                        start=(ci == 0), stop=(ci == NCI - 1),
                    )
                nc.vector.tensor_copy(kT[:, sl], pp[:])
            # v projection per key tile -> vm (+ ones columns for rowsum)
            for kt in range(KT):
                pv = projp.tile([P, P], F32, tag="pv")
                for ci in range(NCI):
                    nc.tensor.matmul(
                        pv[:], xk_sb[ci][:, kt * P : (kt + 1) * P], wv_sb[:, ci, :],
                        start=(ci == 0), stop=(ci == NCI - 1),
                    )
                v = vm[kt]
                nc.vector.tensor_copy(v[:, 0:C], pv[:, 0:C])
                nc.vector.tensor_copy(v[:, C + 1 : 2 * C + 1], pv[:, C : 2 * C])
                nc.gpsimd.memset(v[:, C : C + 1], 1.0)
                nc.gpsimd.memset(v[:, 2 * C + 1 : 2 * C + 2], 1.0)
            # q and gate projections over S columns (both heads at once)
            for ch in range(4):
                pp = projp.tile([P, 512], F32, tag="pp")
                sl = slice(ch * 512, (ch + 1) * 512)
                for ci in range(NCI):
                    nc.tensor.matmul(
                        pp[:], wq_sb[:, ci, :], xt_sb[ci][:, sl],
                        start=(ci == 0), stop=(ci == NCI - 1),
                    )
                nc.vector.tensor_copy(qT[:, sl], pp[:])
            for ch in range(4):
                pp = projp.tile([P, 512], F32, tag="pp")
                sl = slice(ch * 512, (ch + 1) * 512)
                for ci in range(NCI):
                    nc.tensor.matmul(
                        pp[:], wg_sb[:, ci, :], xt_sb[ci][:, sl],
                        start=(ci == 0), stop=(ci == NCI - 1),
                    )
                # sigmoid(v) = 0.5 + 0.5*tanh(v/2); Tanh shares the ACT
                # "exp_and_others" table set with Exp so there is only one
                # table load in the whole kernel.
                nc.scalar.activation(gT[:, sl], pp[:], Tanh, bias=bgv_sb[:], scale=0.5)
                nc.vector.tensor_scalar(gT[:, sl], gT[:, sl], 0.5, 0.5, Alu.mult, Alu.add)

        # ---------------- attention + output projection ----------------
        with (
            tc.tile_pool(name="spsum", bufs=2, space="PSUM") as spsum,
            tc.tile_pool(name="opsum", bufs=1, space="PSUM") as opsum,
            tc.tile_pool(name="posum", bufs=2, space="PSUM") as posum,
            tc.tile_pool(name="ptp", bufs=3) as ptp,
            tc.tile_pool(name="pmp", bufs=2) as pmp,
            tc.tile_pool(name="epi", bufs=4) as epi,
        ):
            def emit_po(qt, tail):
                """Output projection for q-tile qt: needs goun + rrec ready."""
                qsl = slice(qt * P, (qt + 1) * P)
                po0 = posum.tile([P, C_IN], F32, tag="po")
                nc.tensor.matmul(po0[:], goun[0:C, qsl], wo_st[0:C, :],
                                 start=True, stop=True)
                po1 = posum.tile([P, C_IN], F32, tag="po")
                nc.tensor.matmul(po1[:], goun[C:P, qsl], wo_st[C:P, :],
                                 start=True, stop=True)
                t1 = epi.tile([P, C_IN], F32, tag="t1")
                if tail:
                    # no exp pressure left on ACT -> use it for the scale
                    nc.scalar.mul(t1[:], po0[:], rrec[0][:, qt : qt + 1])
                else:
                    nc.vector.tensor_scalar_mul(t1[:], po0[:], rrec[0][:, qt : qt + 1])
                ob = epi.tile([P, C_IN], F32, tag="ob")
                nc.vector.scalar_tensor_tensor(
                    ob[:], po1[:], rrec[1][:, qt : qt + 1], t1[:],
                    Alu.mult, Alu.add,
                )
                nc.sync.dma_start(out_t[qsl, :], ob[:])

            # sections: j-outer so each j-half's output projection becomes
            # ready after two sections and can spread over the next two.
            # The pair bias is injected into PSUM with an identity matmul
            # and the qk matmul accumulates on top; exp reads the finished
            # logits straight out of PSUM. This keeps the PE near-100% busy
            # (the HAM clock gate re-throttles the array to 1.2 GHz if PE
            # duty drops) and leaves DVE/GPSIMD free for the epilogues.
            po_ready = []  # q tiles whose out-projection can be emitted
            gstep = 0
            for j in range(2):
                jsl = slice(j * QH, (j + 1) * QH)
                for h in range(2):
                    rows = slice(C * h, C * (h + 1))
                    op_ = opsum.tile([C + 1, QH], F32, tag="op")
                    for kt in range(KT):
                        emit_eb(gstep + EB_AHEAD)
                        gstep += 1
                        ebt = eb_tiles[(h, j, kt)]
                        sp = spsum.tile([P, QH], F32, tag="sp")
                        # Even k-tiles: raw bias injected on the PE via an
                        # identity matmul, exp reads finished logits from
                        # PSUM. Odd k-tiles: plain qk matmul; the host
                        # pre-exponentiated that bias slice, applied as a
                        # DVE multiply after exp. The even-tile injection
                        # keeps PE duty high enough that the HAM clock gate
                        # holds 2.4 GHz; the odd-tile DVE path keeps two of
                        # six matmuls (and their LDWEIGHTS) off the PE.
                        inject = kt % 2 == 0
                        for ch in range(2):
                            csl = slice(ch * 512, (ch + 1) * 512)
                            if inject:
                                nc.tensor.matmul(
                                    sp[:, csl], ident[:], ebt[:, csl],
                                    start=True, stop=False,
                                )
                            qs = j * QH + ch * 512
                            nc.tensor.matmul(
                                sp[:, csl], kT[rows, kt * P : (kt + 1) * P],
                                qT[rows, qs : qs + 512],
                                start=not inject, stop=True,
                            )
                        pt = ptp.tile([P, QH], BF16, tag="pt")
                        nc.scalar.activation(pt[:], sp[:], Exp)
                        if not inject:
                            pm = pmp.tile([P, QH], BF16, tag="pm")
                            nc.vector.tensor_mul(pm[:], pt[:], ebt[:])
                            pt = pm
                        for ch in range(2):
                            csl = slice(ch * 512, (ch + 1) * 512)
                            nc.tensor.matmul(
                                op_[:, csl],
                                vm[kt][:, (C + 1) * h : (C + 1) * (h + 1)],
                                pt[:, csl],
                                start=(kt == 0), stop=(kt == KT - 1),
                            )
                        # interleave deferred output-projection work
                        if po_ready and kt % 2:
                            emit_po(po_ready.pop(0), tail=False)

                    # section epilogue: rowsum -> rrec columns (DRAM round
                    # trip transposes the row onto partitions), reciprocal,
                    # gate applied to unnormalized o
                    rsum = epi.tile([1, QH], F32, tag="rsum")
                    nc.vector.tensor_copy(rsum[:], op_[C : C + 1, :])
                    dscr = drp.tile([1, QH], F32, tag="dscr")
                    nc.sync.dma_start(dscr[:], rsum[:])
                    rsl = slice((QH // P) * j, (QH // P) * (j + 1))
                    nc.sync.dma_start(
                        rrec[h][:, rsl],
                        dscr[0, :].rearrange("(t p) -> p t", p=P),
                    )
                    nc.vector.reciprocal(rrec[h][:, rsl], rrec[h][:, rsl])
                    nc.vector.tensor_mul(goun[rows, jsl], op_[0:C, :], gT[rows, jsl])
                # goun complete for this j half
                po_ready.extend(range(j * (QH // P), (j + 1) * (QH // P)))
            # emit any remaining output projections (tail); scr3 junk
            # matmuls keep the PE streaming so the HAM clock gate doesn't
            # re-throttle it mid-epilogue
            scr3 = spsum.tile([P, QH], F32, tag="sp")
            for qt in po_ready:
                emit_po(qt, tail=True)
                for _ in range(2):
                    nc.tensor.matmul(scr3[:, 0:512], junk[:, 0:P], junk[:],
                                     start=True, stop=True)

    return nc


_NC_CACHE = {}


def _get_nc(KP):
    if KP not in _NC_CACHE:
        _NC_CACHE[KP] = _build_nc(KP)
    return _NC_CACHE[KP]


def _prepare_core(c, KP, kept, x, bias, attention_mask, Wq, Wk, Wv, Wg, bg, Wo):
    b = c // 4
    h1 = 2 * (c % 4)
    h2 = h1 + 1
    sl1 = slice(h1 * C, (h1 + 1) * C)
    sl2 = slice(h2 * C, (h2 + 1) * C)
    idx = kept[b]
    Kb = len(idx)

    xT = np.ascontiguousarray(x[b].T)  # [C_IN, S]
    xt = xT.reshape(NCI, P, S).astype(NBF)
    xk = np.zeros((C_IN, KP), dtype=np.float32)
    xk[:, :Kb] = xT[:, idx]
    xkt = xk.reshape(NCI, P, KP).astype(NBF)

    def wsel(W, scale=1.0):
        w = np.concatenate([W[:, sl1], W[:, sl2]], axis=1)
        if scale != 1.0:
            w = w * scale
        return np.ascontiguousarray(w.reshape(NCI, P, P)).astype(NBF)

    wq = wsel(Wq, 1.0 / np.sqrt(C))
    wk = wsel(Wk)
    wg = wsel(Wg)
    wv = wsel(Wv)
    bgv = (0.5 * np.concatenate([bg[sl1], bg[sl2]])).reshape(P, 1).astype(np.float32)
    # transposed compacted pair bias: eb[j, kk, q] = bias[b, h_j, q, idx[kk]].
    # Even 128-row k-tiles stay raw (device injects them into the logits on
    # the PE; pads -60 so exp underflows to 0). Odd k-tiles are
    # pre-exponentiated (device applies them as exp(qk)*exp(b); pads 0).
    eb = np.empty((2, KP, S), dtype=NBF)
    for jj, hh in enumerate((h1, h2)):
        bt = np.full((KP, S), -60.0, dtype=np.float32)
        bt[:Kb] = bias[b, hh][:, idx].T
        for kt in range(KP // P):
            blk = bt[kt * P : (kt + 1) * P]
            if kt % 2:
                eb[jj, kt * P : (kt + 1) * P] = np.exp(blk).astype(NBF)
            else:
                eb[jj, kt * P : (kt + 1) * P] = blk.astype(NBF)
    wo = np.concatenate([Wo[sl1, :], Wo[sl2, :]], 0).astype(NBF)

    return {
        "xt": xt,
        "xkt": xkt,
        "wq": wq,
        "wk": wk,
        "wg": wg,
        "wv": wv,
        "bgv": bgv,
        "ebias": eb,
        "wo": wo,
    }


def _run(inputs, trace=False, **kw):
    x = np.asarray(inputs["x"], dtype=np.float32)
    bias = np.asarray(inputs["bias"], dtype=np.float32)
    attention_mask = np.asarray(inputs["attention_mask"])
    Wq = np.asarray(inputs["Wq"], dtype=np.float32)
    Wk = np.asarray(inputs["Wk"], dtype=np.float32)
    Wv = np.asarray(inputs["Wv"], dtype=np.float32)
    Wg = np.asarray(inputs["Wg"], dtype=np.float32)
    bg = np.asarray(inputs["bg"], dtype=np.float32)
    Wo = np.asarray(inputs["Wo"], dtype=np.float32)
    bo = np.asarray(inputs["bo"], dtype=np.float32)

    kept = [np.flatnonzero(attention_mask[b] > 0) for b in range(B)]
    kmax = max(len(k) for k in kept)
    KP = max(256, -(-kmax // 256) * 256)

    in_maps = [None] * 8

    def prep(c):
        in_maps[c] = _prepare_core(
            c, KP, kept, x, bias, attention_mask, Wq, Wk, Wv, Wg, bg, Wo
        )

    threads = [threading.Thread(target=prep, args=(c,)) for c in range(8)]
    for t in threads:
        t.start()
    for t in threads:
        t.join()

    nc = _get_nc(KP)
    res = run_bass_kernel_spmd(nc, in_maps, core_ids=list(range(8)), trace=trace, **kw)

    out = np.empty((B, S, C_IN), dtype=np.float32)
    for b in range(B):
        acc = res.results[4 * b]["out"].astype(np.float32)
        for c in range(4 * b + 1, 4 * b + 4):
            acc = acc + res.results[c]["out"]
        out[b] = acc + bo[None, :]
    return out, res


def kernel(**inputs) -> np.ndarray:
    return _run(inputs)[0]


# revision 27
# speedup vs baseline: 1.0439x; 1.0439x over previous
"""Trainium2 Bass kernel for gated multi-head attention with pair bias.

Reference computation (B=2, S=2048, C_IN=512, H=8, C=64):
    q,k,v = heads(x @ Wq), heads(x @ Wk), heads(x @ Wv)
    logits = q k^T / sqrt(C) + bias + mask_offset
    attn   = softmax(logits)
    o      = attn @ v
    out    = (sigmoid(x @ Wg + bg) * concat(o)) @ Wo + bo

Sharding: 8 cores = 2 batches x 4 head-pairs. Core c handles batch c//4,
heads (2*(c%4), 2*(c%4)+1). Weights are sliced per-core on the host; each
core computes a partial output (sum over its two heads) and the host sums
4 partials per batch and adds bo.

Key-compaction: attention_mask kills ~half the keys exactly (exp(-1e9)=0
in fp32), so the host gathers only the unmasked key positions (padded to
a multiple of 256 -> KP). k/v projections, qk logits, exp, softmax and
attn@v all run on KP instead of S keys, and only the kept columns of the
pair bias are shipped. This is exact, not approximate.

Device math:
  - Transposed orientation throughout (feature dims on partitions).
  - softmax skips max-subtraction (logits are O(+-8)), and uses
    exp(qk) * exp(bias) with exp(bias^T) precomputed on the host in bf16
    (padded key columns get exp-bias 0, which also kills the pad keys).
  - The softmax denominator comes from a ones column appended to v, so
    attn@v produces [o_unnorm ; rowsum] in one accumulation; 1/rowsum is
    applied after the output projection where q sits on partitions.
  - exp(qk)*ebias multiplies are split between DVE and GPSIMD so neither
    engine paces the loop; ACT (exp) is the intended pace-setter.
  - All matmuls bf16 with fp32 PSUM accumulation.
"""

import sys
import threading

import numpy as np

sys.path.insert(0, "/opt/trn_rl_repo")

import ml_dtypes

import concourse.bass as bass
import concourse.tile as tile
from concourse import mybir
from concourse.bass_utils import run_bass_kernel_spmd

# ---------------------------------------------------------------------------
# This toolchain's walrus encodes at most ONE semaphore wait per Drain/CTRL
# instruction; Tile's end-of-kernel drain can carry several (one per DMA
# queue). Split them across a chain of single-wait drains.
# ---------------------------------------------------------------------------


_NOP_UID = [0]


def _split_multi_waits(nc):
    """Rewrite every instruction carrying >1 sem waits: keep one wait on the
    instruction, hoist the others onto same-engine NoOps inserted right
    before it (engine streams execute in order, so this is equivalent)."""
    for fn in nc.m.functions:
        for bb in fn.blocks:
            insts = list(bb.instructions)
            out = []
            changed = False
            for inst in insts:
                si = inst.sync_info
                if si is not None and len(si.on_wait) > 1:
                    changed = True
                    waits = list(si.on_wait)
                    si.on_wait = waits[:1]
                    for w in waits[1:]:
                        _NOP_UID[0] += 1
                        nop = mybir.InstNoOp(
                            name=f"waitsplit-{_NOP_UID[0]}",
                            engine=inst.engine,
                            ins=[],
                            outs=[],
                        )
                        nop.sync_info = mybir.SyncInfo(on_wait=[w], on_update=[])
                        out.append(nop)
                out.append(inst)
            if changed:
                bb.instructions = out


def _drain_and_barrier_split(self, tick_clock, wait_clock):
    from concourse.vector_clock import ScopedClock

    drain_inst = self.nc.sync.drain()
    wait_clock.add_sem_waits(
        drain_inst.ins, ScopedClock({None: tick_clock.global_clock})
    )
    si = drain_inst.ins.sync_info
    if si is not None and len(si.on_wait) > 1:
        extra = list(si.on_wait[1:])
        si.on_wait = list(si.on_wait[:1])
        for w in extra:
            d2 = self.nc.sync.drain()
            d2.ins.sync_info = mybir.SyncInfo(on_wait=[w], on_update=[])

    self.nc.all_engine_barrier()
    assert self.sems is not None
    popped = self.nc._tile_sem_poison_stack.pop()
    assert popped is self._sem_poison
    self.nc.clear_and_free_semaphores(list(self.sems.allocated().values()))
    self.nc.all_engine_barrier()

    _split_multi_waits(self.nc)


tile.TileContext._drain_and_barrier = _drain_and_barrier_split

BF16 = mybir.dt.bfloat16
F32 = mybir.dt.float32
NBF = ml_dtypes.bfloat16

B, S, C_IN, H, C = 2, 2048, 512, 8, 64
P = 128
NCI = C_IN // P  # 4 contraction chunks
QH = 1024  # q-half processed per section
NQT = S // P  # 16 q tiles (output projection)

Exp = mybir.ActivationFunctionType.Exp
Tanh = mybir.ActivationFunctionType.Tanh


def _build_nc(KP):
    """KP = padded compacted key count (multiple of 256)."""
    KT = KP // P  # key tiles
    nc = bass.Bass("TRN2")

    x_t = nc.dram_tensor("xt", [NCI, P, S], BF16, kind="ExternalInput")
    xk_t = nc.dram_tensor("xkt", [NCI, P, KP], BF16, kind="ExternalInput")
    w_q = nc.dram_tensor("wq", [NCI, P, P], BF16, kind="ExternalInput")
    w_k = nc.dram_tensor("wk", [NCI, P, P], BF16, kind="ExternalInput")
    w_g = nc.dram_tensor("wg", [NCI, P, P], BF16, kind="ExternalInput")
    w_v = nc.dram_tensor("wv", [NCI, P, P], BF16, kind="ExternalInput")
    bg_t = nc.dram_tensor("bgv", [P, 1], F32, kind="ExternalInput")
    eb_t = nc.dram_tensor("ebias", [2, KP, S], BF16, kind="ExternalInput")
    wo_t = nc.dram_tensor("wo", [P, C_IN], BF16, kind="ExternalInput")
    out_t = nc.dram_tensor("out", [S, C_IN], F32, kind="ExternalOutput")

    from contextlib import ExitStack

    from concourse.alu_op_type import AluOpType as Alu

    with tile.TileContext(nc) as tc, ExitStack() as ctx:
        const = ctx.enter_context(tc.tile_pool(name="const", bufs=1))
        drp = ctx.enter_context(tc.tile_pool(name="dram", bufs=2, space="DRAM"))

        # ---------------- persistent SBUF tiles + input DMA ----------------
        def load_w(dram):
            t = const.tile([P, NCI, P], BF16, tag=f"w{dram.name}", name=f"wsb_{dram.name}")
            nc.sync.dma_start(t[:], dram[:].rearrange("c p m -> p c m"))
            return t

        # k/v first: their projections start the earliest
        wk_sb, wv_sb = load_w(w_k), load_w(w_v)
        xk_sb = [const.tile([P, KP], BF16, tag=f"xk{i}", name=f"xksb{i}") for i in range(NCI)]
        for i in range(NCI):
            nc.sync.dma_start(xk_sb[i][:], xk_t[i][:])
        wq_sb, wg_sb = load_w(w_q), load_w(w_g)
        xt_sb = [const.tile([P, S], BF16, tag=f"xt{i}", name=f"xtsb{i}") for i in range(NCI)]
        # first q-half of x lands first so the q/g projections (and the first
        # attention section) aren't gated on the whole tensor
        for half in range(2):
            hs = slice(half * QH, (half + 1) * QH)
            for i in range(NCI):
                nc.sync.dma_start(xt_sb[i][:, hs], x_t[i][:, hs])
        bgv_sb = const.tile([P, 1], F32, tag="bgv")
        nc.sync.dma_start(bgv_sb[:], bg_t[:])
        wo_st = const.tile([P, C_IN], BF16, tag="wo")
        nc.sync.dma_start(wo_st[:], wo_t[:])

        from concourse.masks import make_identity

        ident = const.tile([P, P], BF16, tag="ident")
        make_identity(nc, ident[:])
        # matmul fodder for the HAM warm-up burst
        junk = const.tile([P, 512], BF16, tag="junk")
        nc.gpsimd.memset(junk[:], 0.0)

        qT = const.tile([P, S], BF16, tag="qT")
        kT = const.tile([P, KP], BF16, tag="kT")
        gT = const.tile([P, S], BF16, tag="gT")
        goun = const.tile([P, S], BF16, tag="goun")
        vm = [const.tile([P, 2 * (C + 1)], BF16, tag=f"vm{t}", name=f"vm{t}") for t in range(KT)]
        rrec = [const.tile([P, NQT], F32, tag=f"rrec{h}", name=f"rrec{h}") for h in range(2)]

        # exp(bias) tiles stream per (h, j, kt). Triggers are emitted with a
        # fixed lookahead interleaved into the section loops so the sync
        # queue order matches consumption order (emitting them all up front
        # would block the queue on the pool's WAR semaphores and trap the
        # rowsum round-trip DMAs behind them).
        ebp = ctx.enter_context(tc.tile_pool(name="ebp", bufs=5))
        eb_seq = [(h, j, kt) for j in range(2) for h in range(2) for kt in range(KT)]
        eb_tiles = {}
        EB_AHEAD = 3

        def emit_eb(i):
            if i >= len(eb_seq):
                return
            h, j, kt = eb_seq[i]
            t = ebp.tile([P, QH], BF16, tag="eb")
            nc.sync.dma_start(
                t[:], eb_t[h, kt * P : (kt + 1) * P, j * QH : (j + 1) * QH]
            )
            eb_tiles[(h, j, kt)] = t

        for i in range(EB_AHEAD):
            emit_eb(i)

        # ---------------- projections ----------------
        with tc.tile_pool(name="projp", bufs=2, space="PSUM") as projp:
            # HAM warm-up: stream matmuls from t=0 (junk data, result
            # discarded) so the PE clock gate reaches 2.4 GHz before the
            # first real projection instead of ~3.4us into it. 10 MMs
            # cover the initial input-DMA wait.
            scr = projp.tile([P, 512], F32, tag="scr")
            for _ in range(10):
                nc.tensor.matmul(scr[:], junk[:, 0:P], junk[:], start=True, stop=True)
            # k projection over KP columns
            for ch in range(KP // 512):
                pp = projp.tile([P, 512], F32, tag="pp")
                sl = slice(ch * 512, (ch + 1) * 512)
                for ci in range(NCI):
                    nc.tensor.matmul(
                        pp[:], wk_sb[:, ci, :], xk_sb[ci][:, sl],
                        start=(ci == 0), stop=(ci == NCI - 1),
                    )
                nc.vector.tensor_copy(kT[:, sl], pp[:])
            # v projection per key tile -> vm (+ ones columns for rowsum)
            for kt in range(KT):
                pv = projp.tile([P, P], F32, tag="pv")
                for ci in range(NCI):
                    nc.tensor.matmul(
                        pv[:], xk_sb[ci][:, kt * P : (kt + 1) * P], wv_sb[:, ci, :],
                        start=(ci == 0), stop=(ci == NCI - 1),
                    )
                v = vm[kt]
                nc.vector.tensor_copy(v[:, 0:C], pv[:, 0:C])
                nc.vector.tensor_copy(v[:, C + 1 : 2 * C + 1], pv[:, C : 2 * C])
                nc.gpsimd.memset(v[:, C : C + 1], 1.0)
                nc.gpsimd.memset(v[:, 2 * C + 1 : 2 * C + 2], 1.0)
            # q and gate projections over S columns (both heads at once)
            for ch in range(4):
                pp = projp.tile([P, 512], F32, tag="pp")
                sl = slice(ch * 512, (ch + 1) * 512)
                for ci in range(NCI):
                    nc.tensor.matmul(
                        pp[:], wq_sb[:, ci, :], xt_sb[ci][:, sl],
                        start=(ci == 0), stop=(ci == NCI - 1),
                    )
                nc.vector.tensor_copy(qT[:, sl], pp[:])
            for ch in range(4):
                pp = projp.tile([P, 512], F32, tag="pp")
                sl = slice(ch * 512, (ch + 1) * 512)
                for ci in range(NCI):
                    nc.tensor.matmul(
                        pp[:], wg_sb[:, ci, :], xt_sb[ci][:, sl],
                        start=(ci == 0), stop=(ci == NCI - 1),
                    )
                # sigmoid(v) = 0.5 + 0.5*tanh(v/2); Tanh shares the ACT
                # "exp_and_others" table set with Exp so there is only one
                # table load in the whole kernel.
                nc.scalar.activation(gT[:, sl], pp[:], Tanh, bias=bgv_sb[:], scale=0.5)
                nc.vector.tensor_scalar(gT[:, sl], gT[:, sl], 0.5, 0.5, Alu.mult, Alu.add)

        # ---------------- attention + output projection ----------------
        with (
            tc.tile_pool(name="spsum", bufs=2, space="PSUM") as spsum,
            tc.tile_pool(name="opsum", bufs=1, space="PSUM") as opsum,
            tc.tile_pool(name="posum", bufs=2, space="PSUM") as posum,
            tc.tile_pool(name="ptp", bufs=3) as ptp,
            tc.tile_pool(name="epi", bufs=4) as epi,
        ):
            def emit_po(qt, tail):
                """Output projection for q-tile qt: needs goun + rrec ready."""
                qsl = slice(qt * P, (qt + 1) * P)
                po0 = posum.tile([P, C_IN], F32, tag="po")
                nc.tensor.matmul(po0[:], goun[0:C, qsl], wo_st[0:C, :],
                                 start=True, stop=True)
                po1 = posum.tile([P, C_IN], F32, tag="po")
                nc.tensor.matmul(po1[:], goun[C:P, qsl], wo_st[C:P, :],
                                 start=True, stop=True)
                t1 = epi.tile([P, C_IN], F32, tag="t1")
                if tail:
                    # no exp pressure left on ACT -> use it for the scale
                    nc.scalar.mul(t1[:], po0[:], rrec[0][:, qt : qt + 1])
                else:
                    nc.vector.tensor_scalar_mul(t1[:], po0[:], rrec[0][:, qt : qt + 1])
                ob = epi.tile([P, C_IN], F32, tag="ob")
                nc.vector.scalar_tensor_tensor(
                    ob[:], po1[:], rrec[1][:, qt : qt + 1], t1[:],
                    Alu.mult, Alu.add,
                )
                nc.sync.dma_start(out_t[qsl, :], ob[:])

            # sections: j-outer so each j-half's output projection becomes
            # ready after two sections and can spread over the next two.
            # The pair bias is injected into PSUM with an identity matmul
            # and the qk matmul accumulates on top; exp reads the finished
            # logits straight out of PSUM. This keeps the PE near-100% busy
            # (the HAM clock gate re-throttles the array to 1.2 GHz if PE
            # duty drops) and leaves DVE/GPSIMD free for the epilogues.
            po_ready = []  # q tiles whose out-projection can be emitted
            gstep = 0
            for j in range(2):
                jsl = slice(j * QH, (j + 1) * QH)
                for h in range(2):
                    rows = slice(C * h, C * (h + 1))
                    op_ = opsum.tile([C + 1, QH], F32, tag="op")
                    for kt in range(KT):
                        emit_eb(gstep + EB_AHEAD)
                        gstep += 1
                        ebt = eb_tiles[(h, j, kt)]
                        sp = spsum.tile([P, QH], F32, tag="sp")
                        for ch in range(2):
                            csl = slice(ch * 512, (ch + 1) * 512)
                            nc.tensor.matmul(
                                sp[:, csl], ident[:], ebt[:, csl],
                                start=True, stop=False,
                            )
                            qs = j * QH + ch * 512
                            nc.tensor.matmul(
                                sp[:, csl], kT[rows, kt * P : (kt + 1) * P],
                                qT[rows, qs : qs + 512],
                                start=False, stop=True,
                            )
                        pt = ptp.tile([P, QH], BF16, tag="pt")
                        nc.scalar.activation(pt[:], sp[:], Exp)
                        for ch in range(2):
                            csl = slice(ch * 512, (ch + 1) * 512)
                            nc.tensor.matmul(
                                op_[:, csl],
                                vm[kt][:, (C + 1) * h : (C + 1) * (h + 1)],
                                pt[:, csl],
                                start=(kt == 0), stop=(kt == KT - 1),
                            )
                        # interleave deferred output-projection work
                        if po_ready and kt % 2:
                            emit_po(po_ready.pop(0), tail=False)

                    # section epilogue: rowsum -> rrec columns (DRAM round
                    # trip transposes the row onto partitions), reciprocal,
                    # gate applied to unnormalized o
                    rsum = epi.tile([1, QH], F32, tag="rsum")
                    nc.vector.tensor_copy(rsum[:], op_[C : C + 1, :])
                    dscr = drp.tile([1, QH], F32, tag="dscr")
                    nc.sync.dma_start(dscr[:], rsum[:])
                    rsl = slice((QH // P) * j, (QH // P) * (j + 1))
                    nc.sync.dma_start(
                        rrec[h][:, rsl],
                        dscr[0, :].rearrange("(t p) -> p t", p=P),
                    )
                    nc.vector.reciprocal(rrec[h][:, rsl], rrec[h][:, rsl])
                    nc.vector.tensor_mul(goun[rows, jsl], op_[0:C, :], gT[rows, jsl])
                # goun complete for this j half
                po_ready.extend(range(j * (QH // P), (j + 1) * (QH // P)))
            # emit any remaining output projections (tail)
            for qt in po_ready:
                emit_po(qt, tail=True)

    return nc


_NC_CACHE = {}


def _get_nc(KP):
    if KP not in _NC_CACHE:
        _NC_CACHE[KP] = _build_nc(KP)
    return _NC_CACHE[KP]


def _prepare_core(c, KP, kept, x, bias, attention_mask, Wq, Wk, Wv, Wg, bg, Wo):
    b = c // 4
    h1 = 2 * (c % 4)
    h2 = h1 + 1
    sl1 = slice(h1 * C, (h1 + 1) * C)
    sl2 = slice(h2 * C, (h2 + 1) * C)
    idx = kept[b]
    Kb = len(idx)

    xT = np.ascontiguousarray(x[b].T)  # [C_IN, S]
    xt = xT.reshape(NCI, P, S).astype(NBF)
    xk = np.zeros((C_IN, KP), dtype=np.float32)
    xk[:, :Kb] = xT[:, idx]
    xkt = xk.reshape(NCI, P, KP).astype(NBF)

    def wsel(W, scale=1.0):
        w = np.concatenate([W[:, sl1], W[:, sl2]], axis=1)
        if scale != 1.0:
            w = w * scale
        return np.ascontiguousarray(w.reshape(NCI, P, P)).astype(NBF)

    wq = wsel(Wq, 1.0 / np.sqrt(C))
    wk = wsel(Wk)
    wg = wsel(Wg)
    wv = wsel(Wv)
    bgv = (0.5 * np.concatenate([bg[sl1], bg[sl2]])).reshape(P, 1).astype(np.float32)
    # transposed compacted pair bias (raw; injected into the logits on the
    # PE): eb[j, kk, q] = bias[b, h_j, q, idx[kk]]; pad columns get -60 so
    # exp underflows to ~0 and the pad keys vanish from the softmax
    eb = np.full((2, KP, S), -60.0, dtype=NBF)
    for jj, hh in enumerate((h1, h2)):
        eb[jj, :Kb] = bias[b, hh][:, idx].T.astype(NBF)
    wo = np.concatenate([Wo[sl1, :], Wo[sl2, :]], 0).astype(NBF)

    return {
        "xt": xt,
        "xkt": xkt,
        "wq": wq,
        "wk": wk,
        "wg": wg,
        "wv": wv,
        "bgv": bgv,
        "ebias": eb,
        "wo": wo,
    }


def _run(inputs, trace=False, **kw):
    x = np.asarray(inputs["x"], dtype=np.float32)
    bias = np.asarray(inputs["bias"], dtype=np.float32)
    attention_mask = np.asarray(inputs["attention_mask"])
    Wq = np.asarray(inputs["Wq"], dtype=np.float32)
    Wk = np.asarray(inputs["Wk"], dtype=np.float32)
    Wv = np.asarray(inputs["Wv"], dtype=np.float32)
    Wg = np.asarray(inputs["Wg"], dtype=np.float32)
    bg = np.asarray(inputs["bg"], dtype=np.float32)
    Wo = np.asarray(inputs["Wo"], dtype=np.float32)
    bo = np.asarray(inputs["bo"], dtype=np.float32)

    kept = [np.flatnonzero(attention_mask[b] > 0) for b in range(B)]
    kmax = max(len(k) for k in kept)
    KP = max(256, -(-kmax // 256) * 256)

    in_maps = [None] * 8

    def prep(c):
        in_maps[c] = _prepare_core(
            c, KP, kept, x, bias, attention_mask, Wq, Wk, Wv, Wg, bg, Wo
        )

    threads = [threading.Thread(target=prep, args=(c,)) for c in range(8)]
    for t in threads:
        t.start()
    for t in threads:
        t.join()

    nc = _get_nc(KP)
    res = run_bass_kernel_spmd(nc, in_maps, core_ids=list(range(8)), trace=trace, **kw)

    out = np.empty((B, S, C_IN), dtype=np.float32)
    for b in range(B):
        acc = res.results[4 * b]["out"].astype(np.float32)
        for c in range(4 * b + 1, 4 * b + 4):
            acc = acc + res.results[c]["out"]
        out[b] = acc + bo[None, :]
    return out, res


def kernel(**inputs) -> np.ndarray:
    return _run(inputs)[0]
